# revision 1
# baseline (speedup 1.0000x reference)
"""Multi-head causal self-attention (B=2, T=4096, C=768, H=12, D=64) on 8 NeuronCores.

Sharding: core c handles batch b = c // 4 and a group of 3 heads (c % 4).
Each core runs a fused flash-attention pipeline per 512-column tq chunk:
QKV projection -> V transpose -> streaming softmax(QK^T)V -> output projection,
producing a partial (pre-bias) out.T [768, 4096]. The host sums the 4 partials
per batch and adds the projection bias.

Softmax is computed without max-subtraction (scores for this problem are
O(+-6), well inside fp32 exp range); the denominator comes for free from an
appended ones-column in V, so no cross-partition reductions are needed.
All matmuls run as fp32r (PE reduced-precision fp32) with fp32 accumulation.
"""

from contextlib import ExitStack

import numpy as np

import concourse.bass as bass
import concourse.tile as tile
from concourse import bacc
from concourse import mybir
from concourse._compat import with_exitstack
from concourse.bass_utils import run_bass_kernel_spmd

F32 = mybir.dt.float32
F32R = mybir.dt.float32r
BF16 = mybir.dt.bfloat16
EXP = mybir.ActivationFunctionType.Exp
IDENT = mybir.ActivationFunctionType.Identity

B, T, C = 2, 4096, 768
H, D = 12, 64
NCORES = 8
HPC = 3           # heads per core
GPB = NCORES // B  # head-group cores per batch (4)
TQ = 512          # tq chunk width
NJ = T // TQ      # 8
TKB = 128         # tk block
NB = T // TKB     # 32
KC = C // 128     # 6 contraction chunks for the QKV projection
SCALE = 1.0 / np.sqrt(D)

# Layout of the per-core QKV weight columns: 5 chunks of 128 (last half-used).
# Each entry is (quantity, local head, partition base within the chunk).
# Chosen so Q and K of the same head land on the same partition half (their
# zero-padded halves line up in the 128-deep score contraction), and V lands
# where its transpose is convenient.
CHUNKS = [
    [("Q", 0, 0), ("Q", 1, 64)],
    [("Q", 2, 0), ("V", 0, 64)],
    [("K", 0, 0), ("K", 1, 64)],
    [("K", 2, 0), ("V", 1, 64)],
    [("V", 2, 0)],
]
NQKV = 4 * 128 + 64  # 576 columns of per-core qkv weights

QK_BASE = {0: 0, 1: 64, 2: 0}   # partition base of Q/K data per local head
V_BASE = {0: 64, 1: 64, 2: 0}   # partition base of V.T data in its stage tile
ONES_COL = {0: D, 1: D, 2: D}   # denominator ones-column appended after V


def _proj(nc, ps_misc, stp, wp_sb, outT_r, ot, j):
    jsl = slice(j * TQ, (j + 1) * TQ)
    for m in range(KC):
        ps3 = ps_misc.tile([128, TQ], F32, tag="misc", name="ps3")
        for hc in range(HPC):
            nc.tensor.matmul(
                ps3[:],
                lhsT=wp_sb[:, hc, m * 128:(m + 1) * 128],
                rhs=ot[:, hc, :],
                start=(hc == 0),
                stop=(hc == HPC - 1),
            )
        st = stp.tile([128, TQ], F32, tag="st", name="st")
        nc.vector.tensor_copy(st[:], ps3[:])
        nc.sync.dma_start(outT_r[:, m, jsl], st[:])


@with_exitstack
def _mhsa_body(ctx: ExitStack, tc: tile.TileContext, t):
    nc = tc.nc
    xT_r = t["xT"].rearrange("(kc p) t -> p kc t", p=128)
    outT_r = t["outT"].rearrange("(mo p) t -> p mo t", p=128)

    const = ctx.enter_context(tc.tile_pool(name="const", bufs=1))
    persist = ctx.enter_context(tc.tile_pool(name="persist", bufs=1))
    xpool = ctx.enter_context(tc.tile_pool(name="xpool", bufs=2))
    qtp = ctx.enter_context(tc.tile_pool(name="qtp", bufs=2))
    vstp = ctx.enter_context(tc.tile_pool(name="vstp", bufs=2))
    ptp = ctx.enter_context(tc.tile_pool(name="ptp", bufs=6))
    otp = ctx.enter_context(tc.tile_pool(name="otp", bufs=2))
    stp = ctx.enter_context(tc.tile_pool(name="stp", bufs=3))
    lrp = ctx.enter_context(tc.tile_pool(name="lrp", bufs=4))
    rbp = ctx.enter_context(tc.tile_pool(name="rbp", bufs=4))
    dramp = ctx.enter_context(tc.tile_pool(name="dramp", bufs=4, space="DRAM"))

    ps_misc = ctx.enter_context(tc.tile_pool(name="ps_misc", bufs=2, space="PSUM"))
    ps_s = ctx.enter_context(tc.tile_pool(name="ps_s", bufs=2, space="PSUM"))
    ps_o = ctx.enter_context(tc.tile_pool(name="ps_o", bufs=2, space="PSUM"))

    wq_sb = const.tile([128, KC, NQKV], F32R)
    nc.sync.dma_start(wq_sb[:], t["wqkv"].rearrange("(kc p) m -> p kc m", p=128))
    bias_sb = const.tile([128, 5], F32)
    nc.sync.dma_start(bias_sb[:], t["bqkv"].rearrange("m p -> p m"))
    wp_sb = const.tile([128, HPC, C], F32R)
    nc.sync.dma_start(wp_sb[:], t["wproj"].rearrange("h p m -> p h m"))
    id_sb = const.tile([128, 128], F32)
    nc.sync.dma_start(id_sb[:], t["ident"])
    mask_sb = const.tile([128, 1280], BF16)
    nc.sync.dma_start(mask_sb[:], t["masks"])

    KT = [persist.tile([128, T], F32R, tag=f"KT{h}", name=f"KT{h}") for h in range(HPC)]
    Vp = [
        persist.tile([128, NB, D + 1], BF16, tag=f"Vp{h}", name=f"Vp{h}")
        for h in range(HPC)
    ]

    for h in range(HPC):
        pad_lo = 64 - QK_BASE[h]  # 64 if data at 0, 0 if data at 64
        nc.vector.memset(KT[h][pad_lo:pad_lo + 64, :].bitcast(F32), 0.0)
        nc.vector.memset(Vp[h][:, :, ONES_COL[h]:ONES_COL[h] + 1], 1.0)

    for j in range(NJ):
        jsl = slice(j * TQ, (j + 1) * TQ)

        # ---- QKV projection for this tq chunk ----
        xt = xpool.tile([128, KC, TQ], F32R, tag="xt")
        nc.sync.dma_start(xt[:], xT_r[:, :, jsl])
        qt = qtp.tile([128, HPC, TQ], F32R, tag="qt")
        for h in range(HPC):
            pad_lo = 64 - QK_BASE[h]
            nc.vector.memset(qt[pad_lo:pad_lo + 64, h, :].bitcast(F32), 0.0)
        vst = {}
        for m in range(5):
            ents = CHUNKS[m]
            mw = 128 if len(ents) == 2 else 64
            ps = ps_s.tile([128, 2 * TQ], F32, tag="pss", name="ps")[:, :TQ]
            for kc in range(KC):
                nc.tensor.matmul(
                    ps[:mw],
                    lhsT=wq_sb[:, kc, m * 128:m * 128 + mw],
                    rhs=xt[:, kc, :],
                    start=(kc == 0),
                    stop=(kc == KC - 1),
                )
            for (qty, h, base) in ents:
                if qty == "V":
                    vt = vstp.tile([128, TQ], F32, tag=f"vst{h}")
                    vst[h] = vt
                    if mw == 128:
                        # full-tile copy; the other half is junk but finite
                        nc.scalar.activation(
                            out=vt[:], in_=ps[:], func=IDENT,
                            bias=bias_sb[:, m:m + 1],
                        )
                    else:
                        nc.vector.memset(vt[64:128, :], 0.0)
                        nc.scalar.activation(
                            out=vt[0:64, :], in_=ps[0:64, :], func=IDENT,
                            bias=bias_sb[0:64, m:m + 1],
                        )
                else:
                    if qty == "K":
                        dst = KT[h][base:base + 64, jsl]
                    else:
                        dst = qt[base:base + 64, h, :]
                    nc.scalar.activation(
                        out=dst, in_=ps[base:base + 64, :], func=IDENT,
                        bias=bias_sb[base:base + 64, m:m + 1],
                    )

        # ---- V transposes: V.T [64, TQ] stage -> natural V in Vp ----
        for h in range(HPC):
            vt = vst[h]
            vb = V_BASE[h]
            for s in range(4):
                pst = ps_misc.tile([128, TQ], F32, tag="misc")
                nc.tensor.transpose(
                    pst[:, 0:128], vt[:, s * 128:(s + 1) * 128], id_sb[:]
                )
                nc.vector.tensor_copy(
                    out=Vp[h][:, 4 * j + s, 0:D],
                    in_=pst[:, vb:vb + 64],
                )

        # ---- streaming attention for this tq chunk ----
        ot = otp.tile([128, HPC, TQ], F32R, tag="ot")
        for h in range(HPC):
            nc.vector.memset(ot[64:128, h, :].bitcast(F32), 0.0)
        nblk = 4 * j + 4
        npair = nblk // 2
        for h in range(HPC):
            pso = ps_o.tile([128, TQ], F32, tag="pso")
            for ip in range(npair):
                i0, i1 = 2 * ip, 2 * ip + 1
                # column trim offsets: block i only contributes to tq columns
                # >= 128*(i-4j) within this chunk (the rest is fully masked)
                offs = [max(0, 128 * (i - 4 * j)) for i in (i0, i1)]
                ns = [TQ - o for o in offs]
                starts = [0, ns[0]]
                w = ns[0] + ns[1]
                pss = ps_s.tile([128, 2 * TQ], F32, tag="pss")
                for n, i in enumerate((i0, i1)):
                    nc.tensor.matmul(
                        pss[:, starts[n]:starts[n] + ns[n]],
                        lhsT=KT[h][:, i * 128:(i + 1) * 128],
                        rhs=qt[:, h, offs[n]:TQ],
                        start=True,
                        stop=True,
                    )
                pt = ptp.tile([128, 2 * TQ], BF16, tag="pt")
                nc.scalar.activation(
                    out=pt[:, :w], in_=pss[:, :w], func=EXP, scale=SCALE
                )
                rp = ip - 2 * j
                if rp >= 0:  # diagonal pair: packed masks for both blocks
                    moff = 0 if rp == 0 else 896
                    nc.vector.tensor_mul(
                        pt[:, :w], pt[:, :w], mask_sb[:, moff:moff + w]
                    )
                for n, i in enumerate((i0, i1)):
                    nc.tensor.matmul(
                        pso[0:D + 1, offs[n]:TQ],
                        lhsT=Vp[h][:, i, :],
                        rhs=pt[:, starts[n]:starts[n] + ns[n]],
                        start=(i == 0),
                        stop=(i == nblk - 1),
                    )
            # normalize: O.T rows / denominator row (partition D)
            lr = lrp.tile([65, TQ], F32, tag="lr")
            nc.vector.reciprocal(lr[D:D + 1, :], pso[D:D + 1, :])
            ld = dramp.tile([1, TQ], F32, tag="ld")
            nc.sync.dma_start(ld[:], lr[D:D + 1, :])
            rb = rbp.tile([64, TQ], F32, tag="rb")
            nc.sync.dma_start(rb[:], ld[:].to_broadcast((64, TQ)))
            nc.vector.tensor_mul(
                ot[0:64, h, :],
                pso[0:64, :],
                rb[:],
            )

        # ---- output projection, software-pipelined by one chunk ----
        # proj(j-1) is emitted here so the PE has attn(j) work to cover the
        # normalization latency of chunk j-1.
        if j > 0:
            _proj(nc, ps_misc, stp, wp_sb, outT_r, prev_ot, j - 1)
        prev_ot = ot
    _proj(nc, ps_misc, stp, wp_sb, outT_r, prev_ot, NJ - 1)


def build_nc():
    nc = bacc.Bacc("TRN2", target_bir_lowering=False, debug=False)
    t = {}
    t["xT"] = nc.dram_tensor("xT", [C, T], F32R, kind="ExternalInput").ap()
    t["wqkv"] = nc.dram_tensor("wqkv", [C, NQKV], F32R, kind="ExternalInput").ap()
    t["bqkv"] = nc.dram_tensor("bqkv", [5, 128], F32, kind="ExternalInput").ap()
    t["wproj"] = nc.dram_tensor("wproj", [HPC, 128, C], F32R, kind="ExternalInput").ap()
    t["ident"] = nc.dram_tensor("ident", [128, 128], F32, kind="ExternalInput").ap()
    t["masks"] = nc.dram_tensor("masks", [128, 1280], BF16, kind="ExternalInput").ap()
    t["outT"] = nc.dram_tensor("outT", [C, T], F32, kind="ExternalOutput").ap()
    with tile.TileContext(nc) as tc:
        _mhsa_body(tc, t)
    nc.compile()
    return nc


def make_in_maps(x, W_qkv, b_qkv, W_proj):
    """Shard the full inputs into one input map per core."""
    x = np.asarray(x, dtype=np.float32)
    W_qkv = np.asarray(W_qkv, dtype=np.float32)
    b_qkv = np.asarray(b_qkv, dtype=np.float32)
    W_proj = np.asarray(W_proj, dtype=np.float32)

    ident = np.eye(128, dtype=np.float32)
    q_idx = np.arange(TQ)
    p_idx = np.arange(128)
    m4 = np.zeros((4, 128, TQ), dtype=np.float32)
    for r in range(4):
        m4[r] = (p_idx[:, None] <= (q_idx[None, :] - 128 * r)).astype(np.float32)
    import ml_dtypes
    masks = np.concatenate(
        [m4[0], m4[1][:, 128:], m4[2][:, 256:], m4[3][:, 384:]], axis=1
    ).astype(ml_dtypes.bfloat16)  # [128, 512+384+256+128 = 1280]

    in_maps = []
    for c in range(NCORES):
        b = c // GPB
        g = c % GPB
        heads = [HPC * g + h for h in range(HPC)]

        wg = np.zeros((C, NQKV), dtype=np.float32)
        bg = np.zeros((5, 128), dtype=np.float32)
        qty_off = {"Q": 0, "K": C, "V": 2 * C}
        for m, ents in enumerate(CHUNKS):
            for (qty, h, base) in ents:
                src = qty_off[qty] + heads[h] * D
                wg[:, m * 128 + base:m * 128 + base + D] = W_qkv[:, src:src + D]
                bg[m, base:base + D] = b_qkv[src:src + D]

        wp = np.zeros((HPC, 128, C), dtype=np.float32)
        for h in range(HPC):
            wp[h, 0:64] = W_proj[heads[h] * D:(heads[h] + 1) * D, :]

        in_maps.append({
            "xT": np.ascontiguousarray(x[b].T),
            "wqkv": wg,
            "bqkv": bg,
            "wproj": wp,
            "ident": ident,
            "masks": masks,
        })
    return in_maps


def run_cores(inputs, trace=False, **kw):
    nc = build_nc()
    in_maps = make_in_maps(
        inputs["x"], inputs["W_qkv"], inputs["b_qkv"], inputs["W_proj"]
    )
    res = run_bass_kernel_spmd(nc, in_maps, list(range(NCORES)), trace=trace, **kw)
    return res


def gather(results, b_proj):
    out = np.zeros((B, T, C), dtype=np.float32)
    for c in range(NCORES):
        out[c // GPB] += results[c]["outT"].T
    out += np.asarray(b_proj, dtype=np.float32)
    return out


def kernel(x, W_qkv, b_qkv, W_proj, b_proj):
    res = run_cores(
        {"x": x, "W_qkv": W_qkv, "b_qkv": b_qkv, "W_proj": W_proj}
    )
    return gather(res.results, b_proj)

